# revision 18
# baseline (speedup 1.0000x reference)
"""APR tree-level max-pool (segment max over sorted parent_ids) on 8 TRN2 cores.

Strategy
--------
- Shard the 64 (B*C) slices across 8 NeuronCores: 8 slices per core. The
  segment structure (parent_ids) is shared by every slice.
- On the host, build *index* matrices only (no arithmetic on intensities):
  each non-empty segment j becomes one padded row of indices
  [s_j, s_j+1, ..., clamped to e_j-1], bucketed into length classes
  (<=8, <=16, <=Lmax) so padding waste stays small. The host then gathers
  the intensities through those indices (pure data movement / sharding) and
  lays each class out plane-major: X[k, p, f] = element k of row (p, f).
- The device reduces each row with a binary tree of `tensor_max` ops over
  the plane axis (contiguous step-1 bf16 operands -> 2x DVE mode). All the
  actual max arithmetic happens on the NeuronCores.
- bf16 storage/compute: max() commutes with monotone rounding, so the result
  equals the bf16 rounding of the exact f32 max (rel err <= 2^-8, far below
  the 2e-2 gate).
- Host un-permutes the per-class outputs back into segment order and fills
  empty segments with -FLT_MAX, matching the reference.
"""

import numpy as np
import ml_dtypes

B, C, N_IN, N_OUT = 2, 32, 1048576, 131072
N_CORES = 8
SLICES_PER_CORE = (B * C) // N_CORES
FMAX = np.float32(np.finfo(np.float32).max)
BF16 = ml_dtypes.bfloat16

_TILE_ELEMS = 6144  # per-partition bf16 elements of one input tile (~12KB)
_EDGES_SMALL = (2, 4, 6, 8, 10, 12, 14, 16, 20)  # class-width ladder below Lmax
_ALT_ENGINES = True  # alternate in-DMAs between SP and ACT HWDGE engines
_BUFS = 4


def _build_nc(class_shapes, n_iters=1):
    """class_shapes: list of (name, L, Ftot). Returns finalized Bacc graph.

    n_iters > 1 wraps the body in a hardware loop (used only for timing
    experiments; results are identical since the body is idempotent).
    """
    import sys
    if "/opt/trn_rl_repo" not in sys.path:
        sys.path.insert(0, "/opt/trn_rl_repo")
    from concourse import bacc
    import concourse.mybir as mybir
    from concourse.tile import TileContext

    nc = bacc.Bacc(None, target_bir_lowering=False)
    params = {}
    for name, L, Ft in class_shapes:
        x = nc.declare_dram_parameter(f"x{name}", [L, 128, Ft], mybir.dt.bfloat16,
                                      isOutput=False)
        o = nc.declare_dram_parameter(f"o{name}", [128, Ft], mybir.dt.bfloat16,
                                      isOutput=True)
        params[name] = (x, o, L, Ft)

    def emit_body():
        dma_i = 0
        for name, (x, o, L, Ft) in params.items():
            tf = max(4, (_TILE_ELEMS // L) // 4 * 4)
            res = res_pool.tile([128, Ft], mybir.dt.bfloat16, tag=f"res_{name}")
            off = 0
            while off < Ft:
                w = min(tf, Ft - off)
                t = pool.tile([128, L, w], mybir.dt.bfloat16, tag="in")
                if _ALT_ENGINES:
                    eng_in = nc.sync if dma_i % 2 == 0 else nc.scalar
                    eng_out = nc.scalar if dma_i % 2 == 0 else nc.sync
                else:
                    eng_in = eng_out = nc.sync
                dma_i += 1
                eng_in.dma_start(
                    out=t[:],
                    in_=x[:, :, off:off + w].rearrange("l p f -> p l f"),
                )
                cur, h = t, L
                while h > 2:
                    c2 = (h + 1) // 2
                    nxt = pool.tile([128, c2, w], mybir.dt.bfloat16, tag="lvl")
                    nc.vector.tensor_max(
                        nxt[:], cur[:, 0:c2, :], cur[:, h - c2:h, :]
                    )
                    cur, h = nxt, c2
                if h == 2:
                    nc.vector.tensor_max(res[:, off:off + w],
                                         cur[:, 0, :], cur[:, 1, :])
                else:
                    nc.vector.tensor_copy(res[:, off:off + w], cur[:, 0, :])
                off += w
            eng_out = nc.scalar if dma_i % 2 == 0 else nc.sync
            eng_out.dma_start(out=o[:], in_=res[:])

    with TileContext(nc) as tc:
        with tc.tile_pool(name="pool", bufs=_BUFS) as pool, \
             tc.tile_pool(name="res", bufs=1) as res_pool:
            if n_iters > 1:
                with tc.For_i(0, n_iters, 1):
                    emit_body()
            else:
                emit_body()
    nc.finalize()
    return nc


def _prepare(intensities, parent_ids, num_out):
    n_out = int(num_out)
    intens = np.asarray(intensities, dtype=np.float32)
    b, c, n_in = intens.shape
    n_slices = b * c
    data = intens.reshape(n_slices, n_in)
    pid = np.asarray(parent_ids).astype(np.int64)

    counts = np.bincount(pid, minlength=n_out)
    starts = np.zeros(n_out + 1, dtype=np.int64)
    np.cumsum(counts, out=starts[1:])
    maxlen = int(counts.max())

    if maxlen > 512:
        # Far outside the spec's sorted_randint distribution (where
        # maxlen ~ 26); the padded-row layout would waste SBUF. Fall back
        # to a host computation so kernel() stays functional.
        return {"fallback": True, "shape": (b, c, n_out, n_slices),
                "data": data, "counts": counts, "starts": starts}

    # Length classes (rows padded up to the class width); fine classes keep
    # padding waste low. Above the fixed ladder, extend with x1.5 steps so
    # arbitrary length distributions stay within ~1.5x padding. Empty ranges
    # drop out below.
    edges = [e for e in _EDGES_SMALL if e < maxlen]
    e = edges[-1] if edges else 0
    while e < maxlen:
        e = max(e + 2, ((e * 3 // 2) + 1) // 2 * 2)
        edges.append(min(e, ((maxlen + 1) // 2) * 2))
        e = edges[-1]
    bounds = []
    lo = 1
    for e in edges:
        bounds.append((lo, e, e))
        lo = e + 1

    classes = []  # (name, L, ids, Ftot, n_rows)
    data_bf = data.astype(BF16)
    per_core_inputs = [dict() for _ in range(N_CORES)]
    for ci, (lo, hi, L) in enumerate(bounds):
        ids = np.nonzero((counts >= lo) & (counts <= hi))[0]
        if ids.size == 0:
            continue
        name = f"c{ci}"
        lens = counts[ids]
        # [NS, L] clamped indices; duplicates are harmless under max.
        idx = starts[ids][:, None] + np.minimum(
            np.arange(L, dtype=np.int64)[None, :], (lens - 1)[:, None]
        )
        ns = ids.size
        rows_per_core = SLICES_PER_CORE * ns
        ftot = -(-rows_per_core // (128 * 4)) * 4  # pad to multiple of 4
        gathered = data_bf[:, idx.ravel()].reshape(n_slices, ns, L)
        for core in range(N_CORES):
            arr = gathered[core * SLICES_PER_CORE:(core + 1) * SLICES_PER_CORE]
            # [S, NS, L] -> plane-major [L, S*NS]
            arr = arr.transpose(2, 0, 1).reshape(L, rows_per_core)
            full = np.zeros((L, 128 * ftot), dtype=BF16)
            full[:, :rows_per_core] = arr
            per_core_inputs[core][f"x{name}"] = full.reshape(L, 128, ftot)
        classes.append((name, L, ids, ftot, rows_per_core))

    nc = _build_nc([(name, L, ftot) for name, L, ids, ftot, nr in classes])
    return {
        "nc": nc,
        "per_core_inputs": per_core_inputs,
        "classes": classes,
        "shape": (b, c, n_out, n_slices),
    }


def prepare_for_timing(inputs):
    return _prepare(inputs["intensities"], inputs["parent_ids"], inputs["num_out"])


def kernel(intensities, parent_ids, num_out):
    import sys
    if "/opt/trn_rl_repo" not in sys.path:
        sys.path.insert(0, "/opt/trn_rl_repo")
    from concourse.bass_utils import run_bass_kernel_spmd

    prep = _prepare(intensities, parent_ids, num_out)
    b, c, n_out, n_slices = prep["shape"]
    if prep.get("fallback"):
        data, counts, starts = prep["data"], prep["counts"], prep["starts"]
        out = np.full((n_slices, n_out), -FMAX, dtype=np.float32)
        nz = np.nonzero(counts)[0]
        out[:, nz] = np.maximum.reduceat(data, starts[nz], axis=1)
        return out.reshape(b, c, n_out)
    res = run_bass_kernel_spmd(prep["nc"], prep["per_core_inputs"],
                               core_ids=list(range(N_CORES)))

    out = np.full((n_slices, n_out), -FMAX, dtype=np.float32)
    for name, L, ids, ftot, rows_per_core in prep["classes"]:
        for core in range(N_CORES):
            vals = res.results[core][f"o{name}"].reshape(-1)[:rows_per_core]
            vals = vals.reshape(SLICES_PER_CORE, ids.size).astype(np.float32)
            out[core * SLICES_PER_CORE:(core + 1) * SLICES_PER_CORE, ids] = vals
    return out.reshape(b, c, n_out)
